# revision 5
# baseline (speedup 1.0000x reference)
"""Multi-head attention (B=2, D=1024, L=2048, H=16) on 8 TRN2 NeuronCores.

v4 (309.8 us, was 335-347 us): on top of the v1 design --
  - All inputs go through host-prepared layouts that are contiguous per
    SBUF partition (2 KB+ DMA runs instead of 256 B strided pieces), so
    the weight/x streams run at full HWDGE rate and the 14.5 us startup
    stall is gone.
  - Per-pair softmax normalization splits into a vector-side half
    (C stash, denominator pair, reciprocal_approx_fast -- 0.7 us vs
    3.3 us for the exact reciprocal) issued immediately, and a PE-side
    half (selector-broadcast matmul + cn multiply) deferred by one mt so
    the PE never stalls on the serial chain (multi-us PE idle at each mt
    boundary also re-triggered the HAM clock gate, compounding the
    loss).
  - Output projection stays a grouped phase-5 PSUM accumulation
    (interleaving it per-mt as 64 single matmuls + vector adds measured
    30 us WORSE: extra LDWEIGHTS + PSUM open/close + vector pressure).

Sharding: core c handles batch c//4 and query block c%4 (512 queries).
Each core computes K/V projections for its whole batch (duplicated across
the 4 cores sharing a batch -- this avoids any inter-core collective),
attention for its 512 queries over all 16 heads, and the output
projection for its query slice.  Host concatenates the 8 (1024, 512)
slices into the (2, 1024, 2048) output.

Layout choices (per core):
  - Scores are computed transposed: ST[k, q] = sum_d K[d,k] Q[d,q] with
    Lk on partitions, so exp(ST) tiles feed the A@V matmul as the moving
    operand with Lk as the contraction dim.
  - V is produced directly in transposed layout V^T (Lk x DH) by the
    projection out = x_chunk.T @ WvT_chunk, with a ones-column appended
    per head so the A@V matmul also emits the softmax denominator row.
  - Normalization is deferred: unnormalized C and all 16 denominator
    rows are stashed, then one (16, 512) reciprocal + 8 fp32 selector
    matmuls broadcast 1/denom across partitions, one multiply per
    128-row block.  Keeps multi-us serial work off the per-head path so
    the PE never idles long enough for the HAM clock gate to re-throttle.

All matmuls in bf16 (f32 PSUM accumulate); softmax stats in f32.
"""

import sys
import types

import numpy as np
import ml_dtypes


def _install_axon_hooks_shim():
    """antenv.axon_hooks is absent in this image; concourse imports it when
    tracing is requested (e.g. via the BASS_TRACE env var).  Provide the
    module and, if possible, the real NTFF profiling hook so tracing works
    instead of crashing."""
    try:
        import antenv.axon_hooks  # noqa: F401
        return
    except ImportError:
        pass
    try:
        import antenv
    except ImportError:
        return
    mod = types.ModuleType("antenv.axon_hooks")
    mod._hook = None
    mod.set_axon_ntff_profile_hook = lambda h: setattr(mod, "_hook", h)
    mod.get_axon_ntff_profile_hook = lambda: mod._hook
    sys.modules["antenv.axon_hooks"] = mod
    antenv.axon_hooks = mod
    try:
        from trn_agent_boot.trn_boot import _ntff_profile_via_ctypes

        h = _ntff_profile_via_ctypes("/opt/axon/libaxon_pjrt.so")
        if h is not None:
            mod._hook = h
    except Exception:
        pass


_install_axon_hooks_shim()

import concourse.bass as bass
import concourse.mybir as mybir
import concourse.tile as tile
from concourse import bacc
from concourse.bass_utils import run_bass_kernel_spmd
from concourse.tile_rust import add_dep_helper

BF16 = mybir.dt.bfloat16
F32 = mybir.dt.float32
AF = mybir.ActivationFunctionType

B, D, L, H = 2, 1024, 2048, 16
DH = D // H            # 64
P = 128
LQ = L // 4            # 512 queries per core
SCALE = 1.0 / np.sqrt(np.float32(DH))

DC = D // P            # 8 contraction chunks
LT = L // P            # 16 Lk tiles
HV = DH + 1            # V^T per-head width incl. ones column


def build():
    nc = bacc.Bacc(None, target_bir_lowering=False, debug=False)

    # Host-prepared, per-partition-contiguous layouts (see _run): weight
    # chunk slices are 2 KB runs instead of 256 B strided pieces, so the
    # HWDGE streams them at full rate (the baseline's strided loads were
    # the source of its 14.5 us startup stall).
    x = nc.dram_tensor("x", [P, DC, L], BF16, kind="ExternalInput")
    xq = nc.dram_tensor("xq", [P, DC, LQ], BF16, kind="ExternalInput")
    wqc = nc.dram_tensor("wqc", [P, DC, DC, P], BF16, kind="ExternalInput")
    wkc = nc.dram_tensor("wkc", [P, DC, DC, P], BF16, kind="ExternalInput")
    wvc = nc.dram_tensor("wvc", [P, DC, D], BF16, kind="ExternalInput")
    woc = nc.dram_tensor("woc", [P, DC, DC, P], BF16, kind="ExternalInput")
    selp = nc.dram_tensor("selp", [2, P], F32, kind="ExternalInput")
    out = nc.dram_tensor("out", [P, DC, LQ], F32, kind="ExternalOutput")

    with tile.TileContext(nc) as tc:
        with (
            tc.tile_pool(name="consts", bufs=1) as consts,
            tc.tile_pool(name="resident", bufs=1) as res,
            tc.tile_pool(name="wstream", bufs=3) as wpool,
            tc.tile_pool(name="exp", bufs=6) as epool,
            tc.tile_pool(name="norm", bufs=2) as npool,
            tc.tile_pool(name="ps_proj", bufs=2, space="PSUM") as ps_proj,
            tc.tile_pool(name="ps_sc", bufs=2, space="PSUM") as ps_sc,
            tc.tile_pool(name="ps_c", bufs=2, space="PSUM") as ps_c,
        ):
            # ---- small inputs first: xq (sync/HWDGE queue, fast) unblocks
            # the Q projection; bulk loads go on the gpsimd queue. ----
            xq_sb = res.tile([P, DC, LQ], BF16)
            xq_dma = nc.sync.dma_start(out=xq_sb[:], in_=xq[:])
            # selector for per-pair denominator broadcast: selp[j, p] = 1 iff p//64 == j
            selp_sb = consts.tile([2, P], F32)
            nc.sync.dma_start(out=selp_sb[:], in_=selp[:])


            k_sb = res.tile([P, DC, L], BF16)     # K   (D x L)
            q_sb = res.tile([P, DC, LQ], BF16)    # Q   (D x LQ)
            cn_sb = res.tile([P, DC, LQ], BF16)   # normalized C (matmul input)
            vt_sb = res.tile([P, LT, H * HV], BF16)  # V^T tiles + ones cols

            vt4 = vt_sb[:].rearrange("p l (h e) -> p l h e", e=HV)
            nc.vector.memset(vt4[:, :, :, DH : DH + 1], 1.0)

            # ---- Phase 1: Q projection (small, unblocks attention early) ----
            wq_dmas = []
            for mt in range(DC):
                wt = wpool.tile([P, DC, P], BF16, tag="w")
                wq_dmas.append(
                    nc.sync.dma_start(out=wt[:], in_=wqc[:, mt, :, :])
                )
                ps = ps_proj.tile([P, LQ], F32, tag="proj")
                for kt in range(DC):
                    nc.tensor.matmul(
                        ps[:],
                        lhsT=wt[:, kt, :],
                        rhs=xq_sb[:, kt, :],
                        start=(kt == 0),
                        stop=(kt == DC - 1),
                    )
                nc.vector.tensor_copy(out=q_sb[:, mt, :], in_=ps[:])

            # ---- bulk loads: every chunk gated behind the startup-critical
            # xq; xb/wvt interleaved pairwise so the V^T projection can start
            # consuming chunk k as soon as pair k has landed ----
            xb = res.tile([P, DC, L], BF16)       # x[b]  (channels-first)
            wvt_sb = res.tile([P, DC, D], BF16)   # Wv.T resident
            # Wk chunks stream on the scalar queue (also gated behind the
            # startup-critical xq) so the sync queue only carries xq+Wq and
            # the first Q-projection matmul fires ~8 us earlier.
            wk_sb = res.tile([P, DC, DC, P], BF16)
            for mt in range(DC):
                dma = nc.scalar.dma_start(out=wk_sb[:, mt, :, :], in_=wkc[:, mt, :, :])
                add_dep_helper(dma.ins, xq_dma.ins, reason="startup order")
            for kt in range(DC):
                dma = nc.gpsimd.dma_start(out=xb[:, kt, :], in_=x[:, kt, :])
                add_dep_helper(dma.ins, xq_dma.ins, reason="startup order")
                dma = nc.scalar.dma_start(out=wvt_sb[:, kt, :], in_=wvc[:, kt, :])
                add_dep_helper(dma.ins, xq_dma.ins, reason="startup order")

            wo_sb = res.tile([P, DC, DC, P], BF16)
            nc.scalar.dma_start(out=wo_sb[:], in_=woc[:])

            # ---- Phase 2: V^T projection ----
            for lt in range(LT):
                for oc in range(2):
                    ps = ps_proj.tile([P, LQ], F32, tag="proj")
                    for kt in range(DC):
                        nc.tensor.matmul(
                            ps[:],
                            lhsT=xb[:, kt, lt * P : (lt + 1) * P],
                            rhs=wvt_sb[:, kt, oc * 512 : (oc + 1) * 512],
                            start=(kt == 0),
                            stop=(kt == DC - 1),
                        )
                    dest = vt4[:, lt, oc * 8 : (oc + 1) * 8, 0:DH]
                    nc.vector.tensor_copy(
                        out=dest, in_=ps[:].rearrange("p (h e) -> p h e", e=DH)
                    )

            norm_q = []

            def emit_norm_o(item, last=False):
                # deferred by one mt: recip finished during the following
                # mt's attention, so the PE-side broadcast never stalls
                pmt, c_mt, recip = item
                bc_ps = ps_c.tile([P, LQ], F32, tag="c")
                nc.tensor.matmul(
                    bc_ps[:], lhsT=selp_sb[:], rhs=recip[:], start=True,
                    stop=True,
                )
                nc.vector.tensor_mul(
                    out=cn_sb[:, pmt, :], in0=c_mt[:], in1=bc_ps[:]
                )

            # ---- Phase 3: per mt: K projection, then attention for its two
            # heads.  Interleaving keeps the scalar engine (exp) fed while the
            # tensor engine grinds projections, and the two heads' score
            # matmuls (K=64 at partition bases 0 and 64) run concurrently on
            # disjoint PE row groups. ----
            for mt in range(DC):
                wt = wk_sb[:, mt, :, :]
                for ncol in range(L // LQ):
                    ps = ps_proj.tile([P, LQ], F32, tag="proj")
                    for kt in range(DC):
                        nc.tensor.matmul(
                            ps[:],
                            lhsT=wt[:, kt, :],
                            rhs=xb[:, kt, ncol * LQ : (ncol + 1) * LQ],
                            start=(kt == 0),
                            stop=(kt == DC - 1),
                        )
                    nc.vector.tensor_copy(
                        out=k_sb[:, mt, ncol * LQ : (ncol + 1) * LQ], in_=ps[:]
                    )

                # Attention for heads (2mt, 2mt+1).  Both heads' scores for
                # one kt share a single (128, 1024) psum tile: one exp covers
                # both, the pool double-buffers across kt, and the two score
                # matmuls (row groups 0-1 vs 2-3 via partition bases 0/64)
                # issue back-to-back so they run concurrently in the array.
                ha, hb = 2 * mt, 2 * mt + 1
                c_ps_a = ps_c.tile([HV, LQ], F32, tag="c")
                c_ps_b = ps_c.tile([HV, LQ], F32, tag="c")
                for kt in range(LT):
                    s_ab = ps_sc.tile([P, 2 * LQ], F32, tag="sc")
                    nc.tensor.matmul(
                        s_ab[:, 0:LQ],
                        lhsT=k_sb[0:DH, mt, kt * P : (kt + 1) * P],
                        rhs=q_sb[0:DH, mt, :],
                        start=True,
                        stop=True,
                    )
                    nc.tensor.matmul(
                        s_ab[:, LQ : 2 * LQ],
                        lhsT=k_sb[DH:P, mt, kt * P : (kt + 1) * P],
                        rhs=q_sb[DH:P, mt, :],
                        start=True,
                        stop=True,
                    )
                    e_ab = epool.tile([P, 2 * LQ], BF16, tag="e")
                    nc.scalar.activation(e_ab[:], s_ab[:], AF.Exp, scale=float(SCALE))
                    nc.tensor.matmul(
                        c_ps_a[:],
                        lhsT=vt_sb[:, kt, ha * HV : (ha + 1) * HV],
                        rhs=e_ab[:, 0:LQ],
                        start=(kt == 0),
                        stop=(kt == LT - 1),
                    )
                    nc.tensor.matmul(
                        c_ps_b[:],
                        lhsT=vt_sb[:, kt, hb * HV : (hb + 1) * HV],
                        rhs=e_ab[:, LQ : 2 * LQ],
                        start=(kt == 0),
                        stop=(kt == LT - 1),
                    )
                # ---- per-pair normalization: stage both denom rows into a
                # (2, LQ) tile (via DMA: engine APs cannot write partition 1),
                # one reciprocal, one K=2 broadcast matmul (psum slot from the
                # just-released ps_c pool, so projection psum is not starved),
                # one multiply. ----
                # vector-side normalization prep for THIS mt (no PE work):
                # stash C rows to SBUF (freeing PSUM), build the denominator
                # pair, take its fast reciprocal.
                c_mt = npool.tile([P, LQ], F32, tag="cmt")
                den_pair = npool.tile([2, LQ], F32, tag="den")
                for h, c_ps in ((ha, c_ps_a), (hb, c_ps_b)):
                    po = (h % 2) * DH
                    nc.vector.tensor_copy(
                        out=c_mt[po : po + DH, :], in_=c_ps[0:DH, :]
                    )
                    stage = npool.tile([1, LQ], F32, tag="stage")
                    nc.vector.tensor_copy(out=stage[:], in_=c_ps[DH : DH + 1, :])
                    nc.sync.dma_start(
                        out=den_pair[h % 2 : h % 2 + 1, :], in_=stage[:]
                    )
                recip = npool.tile([2, LQ], F32, tag="recip")
                nc.vector.reciprocal_approx_fast(recip[:], den_pair[:])
                norm_q.append((mt, c_mt, recip))
                # PE-side normalization + output projection of the PREVIOUS
                # mt: its reciprocal finished during this mt's attention, so
                # the selector broadcast and the 8 O-matmuls run stall-free
                # (multi-us serial work on the per-head path re-triggers the
                # HAM clock gate; deferring by one mt keeps the PE hot).
                if mt >= 1:
                    emit_norm_o(norm_q.pop(0))

            # ---- Phase 5: output projection (grouped PSUM accumulation).
            # The first group's kt0..6 run while the last pair's reciprocal
            # finishes on the vector engine; its selector broadcast and kt7
            # slot in afterwards, so the PE never stalls on the final norm.
            opool2 = npool  # reuse norm pool for staging tiles
            last_norm = norm_q.pop(0)
            for omt in range(DC):
                ps = ps_proj.tile([P, LQ], F32, tag="proj")
                for kt in range(DC - 1):
                    nc.tensor.matmul(
                        ps[:],
                        lhsT=wo_sb[:, omt, kt, :],
                        rhs=cn_sb[:, kt, :],
                        start=(kt == 0),
                        stop=False,
                    )
                    if omt == 0 and kt == DC - 2:
                        emit_norm_o(last_norm, last=True)
                nc.tensor.matmul(
                    ps[:],
                    lhsT=wo_sb[:, omt, DC - 1, :],
                    rhs=cn_sb[:, DC - 1, :],
                    start=False,
                    stop=True,
                )
                o_sb = opool2.tile([P, LQ], F32, tag="osb")
                nc.vector.tensor_copy(out=o_sb[:], in_=ps[:])
                nc.sync.dma_start(out=out[:, omt, :], in_=o_sb[:])


    if not nc.is_finalized():
        nc.finalize()
    return nc


_NC_CACHE = {}


def _get_nc():
    if "nc" not in _NC_CACHE:
        _NC_CACHE["nc"] = build()
    return _NC_CACHE["nc"]


def _run(x, Wq, Wk, Wv, Wo, trace=False):
    """x: (B, D, L) f32; W*: (D, D) f32. Returns (out, BassKernelResults)."""
    nc = _get_nc()
    bf = ml_dtypes.bfloat16
    xb = np.ascontiguousarray(x).astype(bf)                 # (B, D, L)
    wqt = np.asarray(Wq, np.float32).T.astype(bf)
    wkt = np.asarray(Wk, np.float32).T.astype(bf)
    wvt = np.asarray(Wv, np.float32).T.astype(bf)
    wot = np.asarray(Wo, np.float32).T.astype(bf)

    def prep_w(wt):
        # (D, D) -> (P, DC, DC, P): [kp, mt, ko, j] = wt[ko*128+kp, mt*128+j]
        return np.ascontiguousarray(
            wt.reshape(DC, P, DC, P).transpose(1, 2, 0, 3)
        )

    wqc = prep_w(wqt)
    wkc = prep_w(wkt)
    woc = prep_w(wot)
    wvc = np.ascontiguousarray(wvt.reshape(DC, P, D).transpose(1, 0, 2))
    xc = [
        np.ascontiguousarray(xb[b].reshape(DC, P, L).transpose(1, 0, 2))
        for b in range(B)
    ]

    selp = np.zeros((2, P), np.float32)
    selp[0, 0:DH] = 1.0
    selp[1, DH:P] = 1.0

    in_maps = []
    for c in range(8):
        b = c // 4
        q0 = (c % 4) * LQ
        in_maps.append(
            {
                "x": xc[b],
                "xq": np.ascontiguousarray(xc[b][:, :, q0 : q0 + LQ]),
                "wqc": wqc,
                "wkc": wkc,
                "wvc": wvc,
                "woc": woc,
                "selp": selp,
            }
        )
    res = run_bass_kernel_spmd(nc, in_maps, core_ids=list(range(8)), trace=trace)
    out = np.empty((B, D, L), np.float32)
    for c in range(8):
        b = c // 4
        q0 = (c % 4) * LQ
        oc = res.results[c]["out"]  # (P, DC, LQ): [p, o, l] = C[o*128+p, l]
        out[b][:, q0 : q0 + LQ] = oc.transpose(1, 0, 2).reshape(D, LQ)
    return out, res


def kernel(x, mask, Wq, Wk, Wv, Wo):
    # mask is all-ones by construction (fill: ones) -- softmax over all keys.
    out, _ = _run(x, Wq, Wk, Wv, Wo, trace=False)
    return out



# revision 7
# speedup vs baseline: 1.0206x; 1.0206x over previous
"""Multi-head attention (B=2, D=1024, L=2048, H=16) on 8 TRN2 NeuronCores.

v4 (309.8 us, was 335-347 us): on top of the v1 design --
  - All inputs go through host-prepared layouts that are contiguous per
    SBUF partition (2 KB+ DMA runs instead of 256 B strided pieces), so
    the weight/x streams run at full HWDGE rate and the 14.5 us startup
    stall is gone.
  - Per-pair softmax normalization splits into a vector-side half
    (C stash, denominator pair, reciprocal_approx_fast -- 0.7 us vs
    3.3 us for the exact reciprocal) issued immediately, and a PE-side
    half (selector-broadcast matmul + cn multiply) deferred by one mt so
    the PE never stalls on the serial chain (multi-us PE idle at each mt
    boundary also re-triggered the HAM clock gate, compounding the
    loss).
  - Output projection stays a grouped phase-5 PSUM accumulation
    (interleaving it per-mt as 64 single matmuls + vector adds measured
    30 us WORSE: extra LDWEIGHTS + PSUM open/close + vector pressure).

Sharding: core c handles batch c//4 and query block c%4 (512 queries).
Each core computes K/V projections for its whole batch (duplicated across
the 4 cores sharing a batch -- this avoids any inter-core collective),
attention for its 512 queries over all 16 heads, and the output
projection for its query slice.  Host concatenates the 8 (1024, 512)
slices into the (2, 1024, 2048) output.

Layout choices (per core):
  - Scores are computed transposed: ST[k, q] = sum_d K[d,k] Q[d,q] with
    Lk on partitions, so exp(ST) tiles feed the A@V matmul as the moving
    operand with Lk as the contraction dim.
  - V is produced directly in transposed layout V^T (Lk x DH) by the
    projection out = x_chunk.T @ WvT_chunk, with a ones-column appended
    per head so the A@V matmul also emits the softmax denominator row.
  - Normalization is deferred: unnormalized C and all 16 denominator
    rows are stashed, then one (16, 512) reciprocal + 8 fp32 selector
    matmuls broadcast 1/denom across partitions, one multiply per
    128-row block.  Keeps multi-us serial work off the per-head path so
    the PE never idles long enough for the HAM clock gate to re-throttle.

All matmuls in bf16 (f32 PSUM accumulate); softmax stats in f32.
"""

import sys
import types

import numpy as np
import ml_dtypes


def _install_axon_hooks_shim():
    """antenv.axon_hooks is absent in this image; concourse imports it when
    tracing is requested (e.g. via the BASS_TRACE env var).  Provide the
    module and, if possible, the real NTFF profiling hook so tracing works
    instead of crashing."""
    try:
        import antenv.axon_hooks  # noqa: F401
        return
    except ImportError:
        pass
    try:
        import antenv
    except ImportError:
        return
    mod = types.ModuleType("antenv.axon_hooks")
    mod._hook = None
    mod.set_axon_ntff_profile_hook = lambda h: setattr(mod, "_hook", h)
    mod.get_axon_ntff_profile_hook = lambda: mod._hook
    sys.modules["antenv.axon_hooks"] = mod
    antenv.axon_hooks = mod
    try:
        from trn_agent_boot.trn_boot import _ntff_profile_via_ctypes

        h = _ntff_profile_via_ctypes("/opt/axon/libaxon_pjrt.so")
        if h is not None:
            mod._hook = h
    except Exception:
        pass


_install_axon_hooks_shim()

import concourse.bass as bass
import concourse.mybir as mybir
import concourse.tile as tile
from concourse import bacc
from concourse.bass_utils import run_bass_kernel_spmd
from concourse.tile_rust import add_dep_helper

BF16 = mybir.dt.bfloat16
F32 = mybir.dt.float32
AF = mybir.ActivationFunctionType

B, D, L, H = 2, 1024, 2048, 16
DH = D // H            # 64
P = 128
LQ = L // 4            # 512 queries per core
SCALE = 1.0 / np.sqrt(np.float32(DH))

DC = D // P            # 8 contraction chunks
LT = L // P            # 16 Lk tiles
HV = DH + 1            # V^T per-head width incl. ones column


def build():
    nc = bacc.Bacc(None, target_bir_lowering=False, debug=False)

    # Host-prepared, per-partition-contiguous layouts (see _run): weight
    # chunk slices are 2 KB runs instead of 256 B strided pieces, so the
    # HWDGE streams them at full rate (the baseline's strided loads were
    # the source of its 14.5 us startup stall).
    x = nc.dram_tensor("x", [P, DC, L], BF16, kind="ExternalInput")
    xq = nc.dram_tensor("xq", [P, DC, LQ], BF16, kind="ExternalInput")
    wqc = nc.dram_tensor("wqc", [P, DC, DC, P], BF16, kind="ExternalInput")
    wkc = nc.dram_tensor("wkc", [P, DC, DC, P], BF16, kind="ExternalInput")
    wvc = nc.dram_tensor("wvc", [P, DC, D], BF16, kind="ExternalInput")
    woc = nc.dram_tensor("woc", [P, DC, DC, P], BF16, kind="ExternalInput")
    selp = nc.dram_tensor("selp", [2, P], F32, kind="ExternalInput")
    out = nc.dram_tensor("out", [P, DC, LQ], F32, kind="ExternalOutput")

    with tile.TileContext(nc) as tc:
        with (
            tc.tile_pool(name="consts", bufs=1) as consts,
            tc.tile_pool(name="resident", bufs=1) as res,
            tc.tile_pool(name="wstream", bufs=3) as wpool,
            tc.tile_pool(name="exp", bufs=8) as epool,
            tc.tile_pool(name="norm", bufs=2) as npool,
            tc.tile_pool(name="ps_proj", bufs=2, space="PSUM") as ps_proj,
            tc.tile_pool(name="ps_sc", bufs=2, space="PSUM") as ps_sc,
            tc.tile_pool(name="ps_c", bufs=2, space="PSUM") as ps_c,
        ):
            # ---- small inputs first: xq (sync/HWDGE queue, fast) unblocks
            # the Q projection; bulk loads go on the gpsimd queue. ----
            xq_sb = res.tile([P, DC, LQ], BF16)
            xq_dma = nc.sync.dma_start(out=xq_sb[:], in_=xq[:])
            # selector for per-pair denominator broadcast: selp[j, p] = 1 iff p//64 == j
            selp_sb = consts.tile([2, P], F32)
            nc.sync.dma_start(out=selp_sb[:], in_=selp[:])


            k_sb = res.tile([P, DC, L], BF16)     # K   (D x L)
            q_sb = res.tile([P, DC, LQ], BF16)    # Q   (D x LQ)
            cn_sb = res.tile([P, DC, LQ], BF16)   # normalized C (matmul input)
            vt_sb = res.tile([P, LT, H * HV], BF16)  # V^T tiles + ones cols

            vt4 = vt_sb[:].rearrange("p l (h e) -> p l h e", e=HV)
            nc.vector.memset(vt4[:, :, :, DH : DH + 1], 1.0)

            # ---- Phase 1: Q projection (small, unblocks attention early) ----
            wq_dmas = []
            for mt in range(DC):
                wt = wpool.tile([P, DC, P], BF16, tag="w")
                wq_dmas.append(
                    nc.sync.dma_start(out=wt[:], in_=wqc[:, mt, :, :])
                )
                ps = ps_proj.tile([P, LQ], F32, tag="proj")
                for kt in range(DC):
                    nc.tensor.matmul(
                        ps[:],
                        lhsT=wt[:, kt, :],
                        rhs=xq_sb[:, kt, :],
                        start=(kt == 0),
                        stop=(kt == DC - 1),
                    )
                nc.vector.tensor_copy(out=q_sb[:, mt, :], in_=ps[:])

            # ---- bulk loads: every chunk gated behind the startup-critical
            # xq; xb/wvt interleaved pairwise so the V^T projection can start
            # consuming chunk k as soon as pair k has landed ----
            xb = res.tile([P, DC, L], BF16)       # x[b]  (channels-first)
            wvt_sb = res.tile([P, DC, D], BF16)   # Wv.T resident
            for kt in range(DC):
                dma = nc.gpsimd.dma_start(out=xb[:, kt, :], in_=x[:, kt, :])
                add_dep_helper(dma.ins, xq_dma.ins, reason="startup order")
                dma = nc.scalar.dma_start(out=wvt_sb[:, kt, :], in_=wvc[:, kt, :])
                add_dep_helper(dma.ins, xq_dma.ins, reason="startup order")

            wo_sb = res.tile([P, DC, DC, P], BF16)
            nc.scalar.dma_start(out=wo_sb[:], in_=woc[:])

            # ---- Phase 2: V^T projection ----
            for lt in range(LT):
                for oc in range(2):
                    ps = ps_proj.tile([P, LQ], F32, tag="proj")
                    for kt in range(DC):
                        nc.tensor.matmul(
                            ps[:],
                            lhsT=xb[:, kt, lt * P : (lt + 1) * P],
                            rhs=wvt_sb[:, kt, oc * 512 : (oc + 1) * 512],
                            start=(kt == 0),
                            stop=(kt == DC - 1),
                        )
                    dest = vt4[:, lt, oc * 8 : (oc + 1) * 8, 0:DH]
                    nc.vector.tensor_copy(
                        out=dest, in_=ps[:].rearrange("p (h e) -> p h e", e=DH)
                    )

            norm_q = []

            def emit_norm_o(item, last=False):
                # deferred by one mt: recip finished during the following
                # mt's attention, so the PE-side broadcast never stalls
                pmt, c_mt, recip = item
                bc_ps = ps_c.tile([P, LQ], F32, tag="c")
                nc.tensor.matmul(
                    bc_ps[:], lhsT=selp_sb[:], rhs=recip[:], start=True,
                    stop=True,
                )
                nc.vector.tensor_mul(
                    out=cn_sb[:, pmt, :], in0=c_mt[:], in1=bc_ps[:]
                )

            # ---- Phase 3: per mt: K projection, then attention for its two
            # heads.  Interleaving keeps the scalar engine (exp) fed while the
            # tensor engine grinds projections, and the two heads' score
            # matmuls (K=64 at partition bases 0 and 64) run concurrently on
            # disjoint PE row groups. ----
            for mt in range(DC):
                wt = wpool.tile([P, DC, P], BF16, tag="w")
                nc.sync.dma_start(out=wt[:], in_=wkc[:, mt, :, :])
                for ncol in range(L // LQ):
                    ps = ps_proj.tile([P, LQ], F32, tag="proj")
                    for kt in range(DC):
                        nc.tensor.matmul(
                            ps[:],
                            lhsT=wt[:, kt, :],
                            rhs=xb[:, kt, ncol * LQ : (ncol + 1) * LQ],
                            start=(kt == 0),
                            stop=(kt == DC - 1),
                        )
                    nc.vector.tensor_copy(
                        out=k_sb[:, mt, ncol * LQ : (ncol + 1) * LQ], in_=ps[:]
                    )

                # Attention for heads (2mt, 2mt+1).  Both heads' scores for
                # one kt share a single (128, 1024) psum tile: one exp covers
                # both, the pool double-buffers across kt, and the two score
                # matmuls (row groups 0-1 vs 2-3 via partition bases 0/64)
                # issue back-to-back so they run concurrently in the array.
                ha, hb = 2 * mt, 2 * mt + 1
                c_ps_a = ps_c.tile([HV, LQ], F32, tag="c")
                c_ps_b = ps_c.tile([HV, LQ], F32, tag="c")
                for kt in range(LT):
                    s_ab = ps_sc.tile([P, 2 * LQ], F32, tag="sc")
                    nc.tensor.matmul(
                        s_ab[:, 0:LQ],
                        lhsT=k_sb[0:DH, mt, kt * P : (kt + 1) * P],
                        rhs=q_sb[0:DH, mt, :],
                        start=True,
                        stop=True,
                    )
                    nc.tensor.matmul(
                        s_ab[:, LQ : 2 * LQ],
                        lhsT=k_sb[DH:P, mt, kt * P : (kt + 1) * P],
                        rhs=q_sb[DH:P, mt, :],
                        start=True,
                        stop=True,
                    )
                    e_ab = epool.tile([P, 2 * LQ], BF16, tag="e")
                    nc.scalar.activation(e_ab[:], s_ab[:], AF.Exp, scale=float(SCALE))
                    nc.tensor.matmul(
                        c_ps_a[:],
                        lhsT=vt_sb[:, kt, ha * HV : (ha + 1) * HV],
                        rhs=e_ab[:, 0:LQ],
                        start=(kt == 0),
                        stop=(kt == LT - 1),
                    )
                    nc.tensor.matmul(
                        c_ps_b[:],
                        lhsT=vt_sb[:, kt, hb * HV : (hb + 1) * HV],
                        rhs=e_ab[:, LQ : 2 * LQ],
                        start=(kt == 0),
                        stop=(kt == LT - 1),
                    )
                # ---- per-pair normalization: stage both denom rows into a
                # (2, LQ) tile (via DMA: engine APs cannot write partition 1),
                # one reciprocal, one K=2 broadcast matmul (psum slot from the
                # just-released ps_c pool, so projection psum is not starved),
                # one multiply. ----
                # vector-side normalization prep for THIS mt (no PE work):
                # stash C rows to SBUF (freeing PSUM), build the denominator
                # pair, take its fast reciprocal.
                c_mt = npool.tile([P, LQ], F32, tag="cmt")
                den_pair = npool.tile([2, LQ], F32, tag="den")
                for h, c_ps in ((ha, c_ps_a), (hb, c_ps_b)):
                    po = (h % 2) * DH
                    nc.vector.tensor_copy(
                        out=c_mt[po : po + DH, :], in_=c_ps[0:DH, :]
                    )
                    stage = npool.tile([1, LQ], F32, tag="stage")
                    nc.vector.tensor_copy(out=stage[:], in_=c_ps[DH : DH + 1, :])
                    nc.sync.dma_start(
                        out=den_pair[h % 2 : h % 2 + 1, :], in_=stage[:]
                    )
                recip = npool.tile([2, LQ], F32, tag="recip")
                nc.vector.reciprocal_approx_fast(recip[:], den_pair[:])
                norm_q.append((mt, c_mt, recip))
                # PE-side normalization + output projection of the PREVIOUS
                # mt: its reciprocal finished during this mt's attention, so
                # the selector broadcast and the 8 O-matmuls run stall-free
                # (multi-us serial work on the per-head path re-triggers the
                # HAM clock gate; deferring by one mt keeps the PE hot).
                if mt >= 1:
                    emit_norm_o(norm_q.pop(0))

            # ---- Phase 5: output projection (grouped PSUM accumulation).
            # The first group's kt0..6 matmuls run while the LAST pair's
            # reciprocal finishes on the vector engine; its selector
            # broadcast slots in just before kt7, so the PE never stalls
            # on the final normalization chain. ----
            opool2 = npool  # reuse norm pool for staging tiles
            last_norm = norm_q.pop(0)
            for omt in range(DC):
                ps = ps_proj.tile([P, LQ], F32, tag="proj")
                for kt in range(DC):
                    if omt == 0 and kt == DC - 1:
                        emit_norm_o(last_norm, last=True)
                    nc.tensor.matmul(
                        ps[:],
                        lhsT=wo_sb[:, omt, kt, :],
                        rhs=cn_sb[:, kt, :],
                        start=(kt == 0),
                        stop=(kt == DC - 1),
                    )
                o_sb = opool2.tile([P, LQ], F32, tag="osb")
                nc.vector.tensor_copy(out=o_sb[:], in_=ps[:])
                nc.sync.dma_start(out=out[:, omt, :], in_=o_sb[:])


    if not nc.is_finalized():
        nc.finalize()
    return nc


_NC_CACHE = {}


def _get_nc():
    if "nc" not in _NC_CACHE:
        _NC_CACHE["nc"] = build()
    return _NC_CACHE["nc"]


def _run(x, Wq, Wk, Wv, Wo, trace=False):
    """x: (B, D, L) f32; W*: (D, D) f32. Returns (out, BassKernelResults)."""
    nc = _get_nc()
    bf = ml_dtypes.bfloat16
    xb = np.ascontiguousarray(x).astype(bf)                 # (B, D, L)
    wqt = np.asarray(Wq, np.float32).T.astype(bf)
    wkt = np.asarray(Wk, np.float32).T.astype(bf)
    wvt = np.asarray(Wv, np.float32).T.astype(bf)
    wot = np.asarray(Wo, np.float32).T.astype(bf)

    def prep_w(wt):
        # (D, D) -> (P, DC, DC, P): [kp, mt, ko, j] = wt[ko*128+kp, mt*128+j]
        return np.ascontiguousarray(
            wt.reshape(DC, P, DC, P).transpose(1, 2, 0, 3)
        )

    wqc = prep_w(wqt)
    wkc = prep_w(wkt)
    woc = prep_w(wot)
    wvc = np.ascontiguousarray(wvt.reshape(DC, P, D).transpose(1, 0, 2))
    xc = [
        np.ascontiguousarray(xb[b].reshape(DC, P, L).transpose(1, 0, 2))
        for b in range(B)
    ]

    selp = np.zeros((2, P), np.float32)
    selp[0, 0:DH] = 1.0
    selp[1, DH:P] = 1.0

    in_maps = []
    for c in range(8):
        b = c // 4
        q0 = (c % 4) * LQ
        in_maps.append(
            {
                "x": xc[b],
                "xq": np.ascontiguousarray(xc[b][:, :, q0 : q0 + LQ]),
                "wqc": wqc,
                "wkc": wkc,
                "wvc": wvc,
                "woc": woc,
                "selp": selp,
            }
        )
    res = run_bass_kernel_spmd(nc, in_maps, core_ids=list(range(8)), trace=trace)
    out = np.empty((B, D, L), np.float32)
    for c in range(8):
        b = c // 4
        q0 = (c % 4) * LQ
        oc = res.results[c]["out"]  # (P, DC, LQ): [p, o, l] = C[o*128+p, l]
        out[b][:, q0 : q0 + LQ] = oc.transpose(1, 0, 2).reshape(D, LQ)
    return out, res


def kernel(x, mask, Wq, Wk, Wv, Wo):
    # mask is all-ones by construction (fill: ones) -- softmax over all keys.
    out, _ = _run(x, Wq, Wk, Wv, Wo, trace=False)
    return out



# revision 8
# speedup vs baseline: 1.0279x; 1.0072x over previous
"""Multi-head attention (B=2, D=1024, L=2048, H=16) on 8 TRN2 NeuronCores.

v4 (309.8 us, was 335-347 us): on top of the v1 design --
  - All inputs go through host-prepared layouts that are contiguous per
    SBUF partition (2 KB+ DMA runs instead of 256 B strided pieces), so
    the weight/x streams run at full HWDGE rate and the 14.5 us startup
    stall is gone.
  - Per-pair softmax normalization splits into a vector-side half
    (C stash, denominator pair, reciprocal_approx_fast -- 0.7 us vs
    3.3 us for the exact reciprocal) issued immediately, and a PE-side
    half (selector-broadcast matmul + cn multiply) deferred by one mt so
    the PE never stalls on the serial chain (multi-us PE idle at each mt
    boundary also re-triggered the HAM clock gate, compounding the
    loss).
  - Output projection stays a grouped phase-5 PSUM accumulation
    (interleaving it per-mt as 64 single matmuls + vector adds measured
    30 us WORSE: extra LDWEIGHTS + PSUM open/close + vector pressure).

Sharding: core c handles batch c//4 and query block c%4 (512 queries).
Each core computes K/V projections for its whole batch (duplicated across
the 4 cores sharing a batch -- this avoids any inter-core collective),
attention for its 512 queries over all 16 heads, and the output
projection for its query slice.  Host concatenates the 8 (1024, 512)
slices into the (2, 1024, 2048) output.

Layout choices (per core):
  - Scores are computed transposed: ST[k, q] = sum_d K[d,k] Q[d,q] with
    Lk on partitions, so exp(ST) tiles feed the A@V matmul as the moving
    operand with Lk as the contraction dim.
  - V is produced directly in transposed layout V^T (Lk x DH) by the
    projection out = x_chunk.T @ WvT_chunk, with a ones-column appended
    per head so the A@V matmul also emits the softmax denominator row.
  - Normalization is deferred: unnormalized C and all 16 denominator
    rows are stashed, then one (16, 512) reciprocal + 8 fp32 selector
    matmuls broadcast 1/denom across partitions, one multiply per
    128-row block.  Keeps multi-us serial work off the per-head path so
    the PE never idles long enough for the HAM clock gate to re-throttle.

All matmuls in bf16 (f32 PSUM accumulate); softmax stats in f32.
"""

import sys
import types

import numpy as np
import ml_dtypes


def _install_axon_hooks_shim():
    """antenv.axon_hooks is absent in this image; concourse imports it when
    tracing is requested (e.g. via the BASS_TRACE env var).  Provide the
    module and, if possible, the real NTFF profiling hook so tracing works
    instead of crashing."""
    try:
        import antenv.axon_hooks  # noqa: F401
        return
    except ImportError:
        pass
    try:
        import antenv
    except ImportError:
        return
    mod = types.ModuleType("antenv.axon_hooks")
    mod._hook = None
    mod.set_axon_ntff_profile_hook = lambda h: setattr(mod, "_hook", h)
    mod.get_axon_ntff_profile_hook = lambda: mod._hook
    sys.modules["antenv.axon_hooks"] = mod
    antenv.axon_hooks = mod
    try:
        from trn_agent_boot.trn_boot import _ntff_profile_via_ctypes

        h = _ntff_profile_via_ctypes("/opt/axon/libaxon_pjrt.so")
        if h is not None:
            mod._hook = h
    except Exception:
        pass


_install_axon_hooks_shim()

import concourse.bass as bass
import concourse.mybir as mybir
import concourse.tile as tile
from concourse import bacc
from concourse.bass_utils import run_bass_kernel_spmd
from concourse.tile_rust import add_dep_helper

BF16 = mybir.dt.bfloat16
F32 = mybir.dt.float32
AF = mybir.ActivationFunctionType

B, D, L, H = 2, 1024, 2048, 16
DH = D // H            # 64
P = 128
LQ = L // 4            # 512 queries per core
SCALE = 1.0 / np.sqrt(np.float32(DH))

DC = D // P            # 8 contraction chunks
LT = L // P            # 16 Lk tiles
HV = DH + 1            # V^T per-head width incl. ones column


def build():
    nc = bacc.Bacc(None, target_bir_lowering=False, debug=False)

    # Host-prepared, per-partition-contiguous layouts (see _run): weight
    # chunk slices are 2 KB runs instead of 256 B strided pieces, so the
    # HWDGE streams them at full rate (the baseline's strided loads were
    # the source of its 14.5 us startup stall).
    x = nc.dram_tensor("x", [P, DC, L], BF16, kind="ExternalInput")
    xq = nc.dram_tensor("xq", [P, DC, LQ], BF16, kind="ExternalInput")
    wqc = nc.dram_tensor("wqc", [P, DC, DC, P], BF16, kind="ExternalInput")
    wkc = nc.dram_tensor("wkc", [P, DC, DC, P], BF16, kind="ExternalInput")
    wvc = nc.dram_tensor("wvc", [P, DC, D], BF16, kind="ExternalInput")
    woc = nc.dram_tensor("woc", [P, DC, DC, P], BF16, kind="ExternalInput")
    selp = nc.dram_tensor("selp", [2, P], F32, kind="ExternalInput")
    out = nc.dram_tensor("out", [P, DC, LQ], F32, kind="ExternalOutput")

    with tile.TileContext(nc) as tc:
        with (
            tc.tile_pool(name="consts", bufs=1) as consts,
            tc.tile_pool(name="resident", bufs=1) as res,
            tc.tile_pool(name="wstream", bufs=3) as wpool,
            tc.tile_pool(name="exp", bufs=8) as epool,
            tc.tile_pool(name="norm", bufs=2) as npool,
            tc.tile_pool(name="ps_proj", bufs=2, space="PSUM") as ps_proj,
            tc.tile_pool(name="ps_sc", bufs=2, space="PSUM") as ps_sc,
            tc.tile_pool(name="ps_c", bufs=2, space="PSUM") as ps_c,
        ):
            # ---- small inputs first: xq (sync/HWDGE queue, fast) unblocks
            # the Q projection; bulk loads go on the gpsimd queue. ----
            xq_sb = res.tile([P, DC, LQ], BF16)
            xq_dma = nc.sync.dma_start(out=xq_sb[:], in_=xq[:])
            # selector for per-pair denominator broadcast: selp[j, p] = 1 iff p//64 == j
            selp_sb = consts.tile([2, P], F32)
            nc.sync.dma_start(out=selp_sb[:], in_=selp[:])


            k_sb = res.tile([P, DC, L], BF16)     # K   (D x L)
            q_sb = res.tile([P, DC, LQ], BF16)    # Q   (D x LQ)
            cn_sb = res.tile([P, DC, LQ], BF16)   # normalized C (matmul input)
            vt_sb = res.tile([P, LT, H * HV], BF16)  # V^T tiles + ones cols

            vt4 = vt_sb[:].rearrange("p l (h e) -> p l h e", e=HV)
            nc.vector.memset(vt4[:, :, :, DH : DH + 1], 1.0)

            # ---- Phase 1: Q projection (small, unblocks attention early) ----
            wq_dmas = []
            for mt in range(DC):
                wt = wpool.tile([P, DC, P], BF16, tag="w")
                wq_dmas.append(
                    nc.sync.dma_start(out=wt[:], in_=wqc[:, mt, :, :])
                )
                ps = ps_proj.tile([P, LQ], F32, tag="proj")
                for kt in range(DC):
                    nc.tensor.matmul(
                        ps[:],
                        lhsT=wt[:, kt, :],
                        rhs=xq_sb[:, kt, :],
                        start=(kt == 0),
                        stop=(kt == DC - 1),
                    )
                nc.vector.tensor_copy(out=q_sb[:, mt, :], in_=ps[:])

            # ---- bulk loads: every chunk gated behind the startup-critical
            # xq; xb/wvt interleaved pairwise so the V^T projection can start
            # consuming chunk k as soon as pair k has landed ----
            xb = res.tile([P, DC, L], BF16)       # x[b]  (channels-first)
            wvt_sb = res.tile([P, DC, D], BF16)   # Wv.T resident
            for kt in range(DC):
                dma = nc.gpsimd.dma_start(out=xb[:, kt, :], in_=x[:, kt, :])
                add_dep_helper(dma.ins, xq_dma.ins, reason="startup order")
                dma = nc.scalar.dma_start(out=wvt_sb[:, kt, :], in_=wvc[:, kt, :])
                add_dep_helper(dma.ins, xq_dma.ins, reason="startup order")

            wo_sb = res.tile([P, DC, DC, P], BF16)
            nc.scalar.dma_start(out=wo_sb[:], in_=woc[:])

            # ---- Phase 2: V^T projection ----
            for lt in range(LT):
                for oc in range(2):
                    ps = ps_proj.tile([P, LQ], F32, tag="proj")
                    for kt in range(DC):
                        nc.tensor.matmul(
                            ps[:],
                            lhsT=xb[:, kt, lt * P : (lt + 1) * P],
                            rhs=wvt_sb[:, kt, oc * 512 : (oc + 1) * 512],
                            start=(kt == 0),
                            stop=(kt == DC - 1),
                        )
                    dest = vt4[:, lt, oc * 8 : (oc + 1) * 8, 0:DH]
                    nc.vector.tensor_copy(
                        out=dest, in_=ps[:].rearrange("p (h e) -> p h e", e=DH)
                    )

            norm_q = []

            def emit_norm_o(item, last=False):
                # deferred by one mt: recip finished during the following
                # mt's attention, so the PE-side broadcast never stalls
                pmt, c_mt, recip = item
                bc_ps = ps_c.tile([P, LQ], F32, tag="c")
                nc.tensor.matmul(
                    bc_ps[:], lhsT=selp_sb[:], rhs=recip[:], start=True,
                    stop=True,
                )
                nc.vector.tensor_mul(
                    out=cn_sb[:, pmt, :], in0=c_mt[:], in1=bc_ps[:]
                )

            # ---- Phase 3: per mt: K projection, then attention for its two
            # heads.  Interleaving keeps the scalar engine (exp) fed while the
            # tensor engine grinds projections, and the two heads' score
            # matmuls (K=64 at partition bases 0 and 64) run concurrently on
            # disjoint PE row groups. ----
            for mt in range(DC):
                wt = wpool.tile([P, DC, P], BF16, tag="w")
                nc.sync.dma_start(out=wt[:], in_=wkc[:, mt, :, :])
                for ncol in range(L // LQ):
                    ps = ps_proj.tile([P, LQ], F32, tag="proj")
                    for kt in range(DC):
                        nc.tensor.matmul(
                            ps[:],
                            lhsT=wt[:, kt, :],
                            rhs=xb[:, kt, ncol * LQ : (ncol + 1) * LQ],
                            start=(kt == 0),
                            stop=(kt == DC - 1),
                        )
                    nc.vector.tensor_copy(
                        out=k_sb[:, mt, ncol * LQ : (ncol + 1) * LQ], in_=ps[:]
                    )

                # Attention for heads (2mt, 2mt+1).  Both heads' scores for
                # one kt share a single (128, 1024) psum tile: one exp covers
                # both, the pool double-buffers across kt, and the two score
                # matmuls (row groups 0-1 vs 2-3 via partition bases 0/64)
                # issue back-to-back so they run concurrently in the array.
                ha, hb = 2 * mt, 2 * mt + 1
                c_ps_a = ps_c.tile([HV, LQ], F32, tag="c")
                c_ps_b = ps_c.tile([HV, LQ], F32, tag="c")
                for kt in range(LT):
                    s_ab = ps_sc.tile([P, 2 * LQ], F32, tag="sc")
                    nc.tensor.matmul(
                        s_ab[:, 0:LQ],
                        lhsT=k_sb[0:DH, mt, kt * P : (kt + 1) * P],
                        rhs=q_sb[0:DH, mt, :],
                        start=True,
                        stop=True,
                    )
                    nc.tensor.matmul(
                        s_ab[:, LQ : 2 * LQ],
                        lhsT=k_sb[DH:P, mt, kt * P : (kt + 1) * P],
                        rhs=q_sb[DH:P, mt, :],
                        start=True,
                        stop=True,
                    )
                    e_ab = epool.tile([P, 2 * LQ], BF16, tag="e")
                    nc.scalar.activation(e_ab[:], s_ab[:], AF.Exp, scale=float(SCALE))
                    nc.tensor.matmul(
                        c_ps_a[:],
                        lhsT=vt_sb[:, kt, ha * HV : (ha + 1) * HV],
                        rhs=e_ab[:, 0:LQ],
                        start=(kt == 0),
                        stop=(kt == LT - 1),
                    )
                    nc.tensor.matmul(
                        c_ps_b[:],
                        lhsT=vt_sb[:, kt, hb * HV : (hb + 1) * HV],
                        rhs=e_ab[:, LQ : 2 * LQ],
                        start=(kt == 0),
                        stop=(kt == LT - 1),
                    )
                # ---- per-pair normalization: stage both denom rows into a
                # (2, LQ) tile (via DMA: engine APs cannot write partition 1),
                # one reciprocal, one K=2 broadcast matmul (psum slot from the
                # just-released ps_c pool, so projection psum is not starved),
                # one multiply. ----
                # vector-side normalization prep for THIS mt (no PE work):
                # stash C rows to SBUF (freeing PSUM), build the denominator
                # pair, take its fast reciprocal.
                c_mt = npool.tile([P, LQ], F32, tag="cmt")
                den_pair = npool.tile([2, LQ], F32, tag="den")
                for h, c_ps in ((ha, c_ps_a), (hb, c_ps_b)):
                    po = (h % 2) * DH
                    nc.vector.tensor_copy(
                        out=c_mt[po : po + DH, :], in_=c_ps[0:DH, :]
                    )
                    stage = npool.tile([1, LQ], F32, tag="stage")
                    nc.vector.tensor_copy(out=stage[:], in_=c_ps[DH : DH + 1, :])
                    nc.sync.dma_start(
                        out=den_pair[h % 2 : h % 2 + 1, :], in_=stage[:]
                    )
                recip = npool.tile([2, LQ], F32, tag="recip")
                nc.vector.reciprocal_approx_fast(recip[:], den_pair[:])
                norm_q.append((mt, c_mt, recip))
                # PE-side normalization + output projection of the PREVIOUS
                # mt: its reciprocal finished during this mt's attention, so
                # the selector broadcast and the 8 O-matmuls run stall-free
                # (multi-us serial work on the per-head path re-triggers the
                # HAM clock gate; deferring by one mt keeps the PE hot).
                if mt >= 1:
                    emit_norm_o(norm_q.pop(0))
                if mt == DC - 1:
                    emit_norm_o(norm_q.pop(0), last=True)

            # ---- Phase 5: output projection (grouped PSUM accumulation) ----
            opool2 = npool  # reuse norm pool for staging tiles
            for omt in range(DC):
                ps = ps_proj.tile([P, LQ], F32, tag="proj")
                for kt in range(DC):
                    nc.tensor.matmul(
                        ps[:],
                        lhsT=wo_sb[:, omt, kt, :],
                        rhs=cn_sb[:, kt, :],
                        start=(kt == 0),
                        stop=(kt == DC - 1),
                    )
                o_sb = opool2.tile([P, LQ], F32, tag="osb")
                nc.vector.tensor_copy(out=o_sb[:], in_=ps[:])
                nc.sync.dma_start(out=out[:, omt, :], in_=o_sb[:])


    if not nc.is_finalized():
        nc.finalize()
    return nc


_NC_CACHE = {}


def _get_nc():
    if "nc" not in _NC_CACHE:
        _NC_CACHE["nc"] = build()
    return _NC_CACHE["nc"]


def _run(x, Wq, Wk, Wv, Wo, trace=False):
    """x: (B, D, L) f32; W*: (D, D) f32. Returns (out, BassKernelResults)."""
    nc = _get_nc()
    bf = ml_dtypes.bfloat16
    xb = np.ascontiguousarray(x).astype(bf)                 # (B, D, L)
    wqt = np.asarray(Wq, np.float32).T.astype(bf)
    wkt = np.asarray(Wk, np.float32).T.astype(bf)
    wvt = np.asarray(Wv, np.float32).T.astype(bf)
    wot = np.asarray(Wo, np.float32).T.astype(bf)

    def prep_w(wt):
        # (D, D) -> (P, DC, DC, P): [kp, mt, ko, j] = wt[ko*128+kp, mt*128+j]
        return np.ascontiguousarray(
            wt.reshape(DC, P, DC, P).transpose(1, 2, 0, 3)
        )

    wqc = prep_w(wqt)
    wkc = prep_w(wkt)
    woc = prep_w(wot)
    wvc = np.ascontiguousarray(wvt.reshape(DC, P, D).transpose(1, 0, 2))
    xc = [
        np.ascontiguousarray(xb[b].reshape(DC, P, L).transpose(1, 0, 2))
        for b in range(B)
    ]

    selp = np.zeros((2, P), np.float32)
    selp[0, 0:DH] = 1.0
    selp[1, DH:P] = 1.0

    in_maps = []
    for c in range(8):
        b = c // 4
        q0 = (c % 4) * LQ
        in_maps.append(
            {
                "x": xc[b],
                "xq": np.ascontiguousarray(xc[b][:, :, q0 : q0 + LQ]),
                "wqc": wqc,
                "wkc": wkc,
                "wvc": wvc,
                "woc": woc,
                "selp": selp,
            }
        )
    res = run_bass_kernel_spmd(nc, in_maps, core_ids=list(range(8)), trace=trace)
    out = np.empty((B, D, L), np.float32)
    for c in range(8):
        b = c // 4
        q0 = (c % 4) * LQ
        oc = res.results[c]["out"]  # (P, DC, LQ): [p, o, l] = C[o*128+p, l]
        out[b][:, q0 : q0 + LQ] = oc.transpose(1, 0, 2).reshape(D, LQ)
    return out, res


def kernel(x, mask, Wq, Wk, Wv, Wo):
    # mask is all-ones by construction (fill: ones) -- softmax over all keys.
    out, _ = _run(x, Wq, Wk, Wv, Wo, trace=False)
    return out

